# revision 3
# baseline (speedup 1.0000x reference)
"""DenseCapsule dynamic-routing kernel for 8 Trainium2 NeuronCores.

Problem: x[B=32,I=2048,D=16], w_ij[J=64,I=2048,C=32,D=16]
  u_hat = einsum('bid,jicd->bjic', x, w_ij)
  5 routing iterations (softmax over J, s = sum_i c*u_hat, v = squash(s),
  b += sum_c v*u_hat), return v [B,J,C].

Sharding: input capsules I are split 8 ways (I_LOC=256 per core).  The
softmax over J is core-local; the only collective is an AllReduce of the
per-core partial s [B,JC] once per iteration (fp16, in 4 j-chunks so the
AR flight overlaps the per-chunk squash).

Layout: u_hat tiles are [128=(g4,b32) partitions, (c32, j64) free] — c
OUTER, j inner.  With c outermost both per-(b,j,i) weight applications
(v for the logit update, c_ij for the s-sum) broadcast into the DVE
multiply as stride-0 non-innermost APs, keeping the fp16 2x mode — no
ACT broadcast pass.  The c-reduction for the logit update is a tree of
contiguous-half adds.  Softmax runs in f32 without max subtraction
(logits stay < ~40, safe in f32); 1/Z is folded into the f32->fp16
conversion of e, so the s-matmul stationary is the constant block-diag
d1 for the whole kernel.
"""

import numpy as np

B, I, D, J, C = 32, 2048, 16, 64, 32
NCORES = 8
I_LOC = I // NCORES      # 256
G = 4                    # i's per block (G*D = 64 contraction partitions)
NBLK = I_LOC // G        # 64
JC = J * C               # 2048
ITERS = 5
EPS = 1e-7
NCH = 4                  # 512-wide flat matmul chunks over (c,j)
GR = 4                   # i-blocks per phase-2 tile group
NGRP = NBLK // GR        # 16
NJCH = 4                 # j-chunks for the AllReduce pipeline
JCH = J // NJCH          # 16 j's per chunk

_CACHE = {}


def _build():
    import concourse.bacc as bacc
    import concourse.mybir as mybir
    from concourse import tile

    f32 = mybir.dt.float32
    fp16 = mybir.dt.float16
    Act = mybir.ActivationFunctionType
    Alu = mybir.AluOpType

    nc = bacc.Bacc("TRN2", target_bir_lowering=False, debug=False,
                   num_devices=NCORES)
    xd = nc.dram_tensor("xd", [NBLK, G * D, 128], fp16, kind="ExternalInput").ap()
    wm = nc.dram_tensor("wm", [NBLK, G * D, JC], fp16, kind="ExternalInput").ap()
    d1 = nc.dram_tensor("d1", [128, B], fp16, kind="ExternalInput").ap()
    v_out = nc.dram_tensor("v_out", [B, JC], f32, kind="ExternalOutput").ap()

    with tile.TileContext(nc) as tc:
        with tc.tile_pool(name="const", bufs=1) as constp, \
             tc.tile_pool(name="io", bufs=2) as iop, \
             tc.tile_pool(name="u", bufs=3) as up, \
             tc.tile_pool(name="work", bufs=2) as wp, \
             tc.tile_pool(name="small", bufs=1) as sp, \
             tc.tile_pool(name="spg", bufs=3) as spg, \
             tc.tile_pool(name="psum", bufs=4, space="PSUM") as pp, \
             tc.tile_pool(name="spsum", bufs=1, space="PSUM") as spp, \
             tc.tile_pool(name="ud", bufs=1, space="DRAM") as udp, \
             tc.tile_pool(name="ar", bufs=2, space="DRAM") as arp:

            d1_t = constp.tile([128, B], fp16)
            nc.sync.dma_start(d1_t[:], d1[:])
            b_tiles = []                                 # routing logits, f32
            for g in range(NGRP):
                bt = constp.tile([128, GR, J], f32, tag=f"b{g}")
                nc.gpsimd.memset(bt[:], 0.0)
                b_tiles.append(bt)
            u_store = udp.tile([NBLK, 128, JC], fp16)
            v_rep = constp.tile([128, JC], fp16, tag="v_rep")

            # ---- Phase 1: u_hat production + iteration-1 s accumulation
            s_ps = spp.tile([B, JC], f32, tag="s")
            for blk in range(NBLK):
                xd_t = iop.tile([G * D, 128], fp16, tag="xd_t")
                nc.sync.dma_start(xd_t[:], xd[blk])
                wm_t = iop.tile([G * D, JC], fp16, tag="wm_t")
                nc.sync.dma_start(wm_t[:], wm[blk])
                u16 = iop.tile([128, JC], fp16, tag="u16")
                for ch in range(NCH):
                    sl = slice(ch * 512, (ch + 1) * 512)
                    ps = pp.tile([128, 512], f32, tag="ps")
                    nc.tensor.matmul(ps[:], xd_t[:], wm_t[:, sl],
                                     start=True, stop=True)
                    if ch < 2:
                        nc.vector.tensor_copy(u16[:, sl], ps[:])
                    elif ch == 2 or blk % 2 == 0:
                        nc.scalar.copy(u16[:, sl], ps[:])
                    else:
                        nc.vector.tensor_copy(u16[:, sl], ps[:])
                nc.sync.dma_start(u_store[blk], u16[:])
                for ch in range(NCH):
                    sl = slice(ch * 512, (ch + 1) * 512)
                    nc.tensor.matmul(s_ps[:, sl], d1_t[:], u16[:, sl],
                                     start=(blk == 0), stop=(blk == NBLK - 1))

            # ---- Phase 2: routing iterations
            for it in range(1, ITERS + 1):
                # --- AllReduce of s in 4 j-chunks (fp16), squash per chunk
                s3 = s_ps[:].rearrange("p (c j) -> p c j", j=J)
                v16_full = sp.tile([B, C, J], fp16, tag="v16")
                v32_full = sp.tile([B, C, J], f32, tag="v32")
                vr4 = v_rep[:].rearrange("p (c j) -> p c j", j=J)
                for ch in range(NJCH):
                    jsl = slice(ch * JCH, (ch + 1) * JCH)
                    s16 = spg.tile([B, C, JCH], fp16, tag="s16")
                    nc.vector.tensor_scalar_mul(
                        s16[:], s3[:, :, jsl],
                        (1.0 / J) if it == 1 else 1.0)
                    ar_in = arp.tile([B, C * JCH], fp16, tag=f"ari{ch}")
                    ar_out = arp.tile([B, C * JCH], fp16, tag=f"aro{ch}")
                    nc.sync.dma_start(
                        ar_in[:].rearrange("p (c j) -> p c j", j=JCH), s16[:])
                    nc.gpsimd.collective_compute(
                        "AllReduce", Alu.add,
                        replica_groups=[list(range(NCORES))],
                        ins=[ar_in.opt()], outs=[ar_out.opt()],
                    )
                    sf = spg.tile([B, C, JCH], fp16, tag="sf")
                    nc.sync.dma_start(
                        sf[:], ar_out[:].rearrange("p (c j) -> p c j", j=JCH))
                    # squash (f32): v0 = s+eps; scale = sqrt(n)/(1+n)
                    v0 = spg.tile([B, C, JCH], f32, tag="v0")
                    nc.vector.tensor_scalar_add(v0[:], sf[:], EPS)
                    sq = spg.tile([B, C, JCH], f32, tag="sq")
                    nc.vector.tensor_mul(sq[:], v0[:], v0[:])
                    for tw in (16, 8, 4, 2):
                        nc.vector.tensor_add(sq[:, 0:tw, :], sq[:, 0:tw, :],
                                             sq[:, tw:2 * tw, :])
                    norm = spg.tile([B, 1, JCH], f32, tag="nm")
                    nc.vector.tensor_add(norm[:], sq[:, 0:1, :], sq[:, 1:2, :])
                    rt = spg.tile([B, 1, JCH], f32, tag="rt")
                    nc.scalar.activation(rt[:], norm[:], Act.Sqrt)
                    np1 = spg.tile([B, 1, JCH], f32, tag="np")
                    nc.vector.tensor_scalar_add(np1[:], norm[:], 1.0)
                    inv1 = spg.tile([B, 1, JCH], f32, tag="iv")
                    nc.vector.reciprocal(inv1[:], np1[:])
                    invd = spg.tile([B, 1, JCH], f32, tag="id")
                    nc.vector.tensor_mul(invd[:], rt[:], inv1[:])
                    vdst = v32_full if it == ITERS else v16_full
                    nc.vector.tensor_mul(
                        vdst[:, :, jsl], v0[:],
                        invd[:].broadcast_to((B, C, JCH)))
                    if it < ITERS:
                        for g in range(G):
                            nc.sync.dma_start(
                                vr4[g * B:(g + 1) * B, :, jsl],
                                v16_full[:, :, jsl])

                if it == ITERS:
                    nc.sync.dma_start(
                        v_out[:], v32_full[:].rearrange("p c j -> p (c j)"))
                    break

                s_ps = spp.tile([B, JC], f32, tag="s")
                vb = v_rep[:] \
                    .rearrange("p (o c j) -> p o c j", o=1, j=J) \
                    .broadcast_to((128, GR, C, J))

                def stage_b(u_t, c16, g0):
                    # prod2 = u * c_rep (c broadcast over the outer axis),
                    # then the s-matmuls with constant stationary d1.
                    cb = c16[:].rearrange("p n (o j) -> p n o j", o=1) \
                        .broadcast_to((128, GR, C, J))
                    prod2 = wp.tile([128, GR, C, J], fp16, tag="prod1",
                                    bufs=3)
                    nc.vector.tensor_mul(prod2[:], u_t[:], cb)
                    p2f = prod2[:].rearrange("p n c j -> p n (c j)")
                    for n in range(GR):
                        blk = g0 + n
                        for ch in range(NCH):
                            sl = slice(ch * 512, (ch + 1) * 512)
                            nc.tensor.matmul(s_ps[:, ch * 512:(ch + 1) * 512],
                                             d1_t[:], p2f[:, n, sl],
                                             start=(blk == 0),
                                             stop=(blk == NBLK - 1))

                pending = None
                for g in range(NGRP):
                    g0 = g * GR
                    b_g = b_tiles[g]
                    u_t = up.tile([128, GR, C, J], fp16, tag="u_t", bufs=3)
                    nc.sync.dma_start(
                        u_t[:].rearrange("p n c j -> p n (c j)"),
                        u_store[g0:g0 + GR].rearrange("n p f -> p n f"))
                    # logit update t = sum_c u*v via in-place fp16 tree of
                    # contiguous-half adds over the outer c axis.
                    prod1 = wp.tile([128, GR, C, J], fp16, tag="prod1",
                                    bufs=3)
                    nc.vector.tensor_mul(prod1[:], u_t[:], vb)
                    for tw in (16, 8, 4, 2):
                        nc.vector.tensor_add(
                            prod1[:, :, 0:tw, :], prod1[:, :, 0:tw, :],
                            prod1[:, :, tw:2 * tw, :])
                    t_grp = spg.tile([128, GR, 1, J], fp16, tag="t_grp")
                    nc.vector.tensor_add(t_grp[:], prod1[:, :, 0:1, :],
                                         prod1[:, :, 1:2, :])
                    nc.gpsimd.tensor_add(
                        b_g[:], b_g[:],
                        t_grp[:].rearrange("p n o j -> p n (o j)"))
                    # softmax over j: f32 exp (no max), Z per (partition, n),
                    # 1/Z folded into the fp16 conversion of e.
                    e_g = spg.tile([128, GR, J], f32, tag="e_g", bufs=2)
                    nc.scalar.activation(e_g[:], b_g[:], Act.Exp)
                    zr = spg.tile([128, GR], f32, tag="zr")
                    nc.vector.reduce_sum(zr[:], e_g[:],
                                         axis=mybir.AxisListType.X)
                    iz = spg.tile([128, GR], f32, tag="iz")
                    nc.vector.reciprocal(iz[:], zr[:])
                    c16 = spg.tile([128, GR, J], fp16, tag="c16", bufs=2)
                    for n in range(GR):
                        nc.vector.tensor_scalar_mul(
                            c16[:, n, :], e_g[:, n, :], iz[:, n:n + 1])
                    # software pipeline: emit the previous group's prod2 +
                    # matmuls after this group's A-stage so DVE/PE have
                    # ready work while this group's softmax chain runs.
                    if pending is not None:
                        stage_b(*pending)
                    pending = (u_t, c16, g0)
                stage_b(*pending)

    nc.compile()
    return nc


def _prep_inputs(x, w_ij):
    """Host-side shard + layout. Returns per-core in_maps."""
    x_t = np.ascontiguousarray(x.transpose(1, 2, 0)).astype(np.float16)   # [I,D,B]
    # (c,j)-ordered weights: [I, D, C, J]
    w_t = np.ascontiguousarray(w_ij.transpose(1, 3, 2, 0)).astype(np.float16)
    d1 = np.tile(np.eye(B, dtype=np.float16), (G, 1))                     # [128,B]
    in_maps = []
    for k in range(NCORES):
        xs = x_t[k * I_LOC:(k + 1) * I_LOC].reshape(NBLK, G, D, B)
        xdv = np.zeros((NBLK, G * D, 128), np.float16)
        for g in range(G):
            xdv[:, g * D:(g + 1) * D, g * B:(g + 1) * B] = xs[:, g]
        ws = w_t[k * I_LOC:(k + 1) * I_LOC].reshape(NBLK, G * D, JC)
        in_maps.append({"xd": xdv, "wm": np.ascontiguousarray(ws), "d1": d1})
    return in_maps


def kernel(x, w_ij, _trace=False):
    from concourse import bass_utils

    if "nc" not in _CACHE:
        _CACHE["nc"] = _build()
    nc = _CACHE["nc"]
    in_maps = _prep_inputs(np.asarray(x), np.asarray(w_ij))
    res = bass_utils.run_bass_kernel_spmd(
        nc, in_maps, core_ids=list(range(NCORES)), trace=_trace)
    _CACHE["last_result"] = res
    # v_out is [B, (c,j)] — reorder to [B, J, C]
    v = res.results[0]["v_out"].reshape(B, C, J).transpose(0, 2, 1)
    return np.ascontiguousarray(v.astype(np.float32))
